# revision 47
# baseline (speedup 1.0000x reference)
"""Single-head causal attention (B=4, S=4096, d_e=512, d_k=d_v=64) on 8 TRN2 cores.

SPMD: one program on all 8 cores; per-core behavior driven purely by input data.
  - core c handles batch b=c//2; the two cores of a batch split the 8 q-tiles
    (512 queries each) load-balanced: parity 0 -> q-tiles {0,2,5,7}, parity 1 ->
    {1,3,4,6} (equal causal work: 18 kv-tile interactions each, padded to 20 --
    provably minimal for any SPMD-uniform per-position pair budget).
  - All PE/DVE data is bf16 (halves HBM + on-chip traffic; PSUM stays f32).
    The steady state is ACT-exp-throughput-bound (~1.04us per 256kv x 512q
    score pair), so the whole kernel is built around keeping the exp stream
    dense: a single flat 40-pair pipeline across group boundaries, projections
    (kv 4-matmul chunks, v-transposes, q) back-loaded into late exp-heavy
    groups, and the kv projection split into matmul+copy vs transpose phases
    so the vts round-trip hides behind score pairs.
  - Attention in "scores^T" layout: st[s,q] = k @ (q/sqrt(dk))^T. The softmax
    denominator rides the AV matmul via an appended ones column on v (vaug
    blocks are 66-wide so bf16 PSUM transposes stay 4-byte aligned). Causal /
    padding masks are multiplicative {0,1} bf16 tiles picked from an SBUF
    palette at data-driven dynamic offsets (2 base register offsets per group,
    block shifts are static -128*b arithmetic), applied post-exp on DVE with
    each masked pair's AV matmuls deferred exactly one flush slot.
  - DMA issues cost ~1.5us of sequencer time each, so they are kept off the
    ACT queue past the first 4 (ACT dispatches exp) and spread SP/ACT in
    landing-priority order; moff/ident ride Pool's software DGE.
  - Output leaves the device unnormalized as bf16 [65, 512] per group (64 AV
    rows + denominator row); the host divides and transposes during assembly.

The PE is kept busy with scratch matmuls during the initial DMA wait so
the pstate ramp (0.65->1.2->2.4GHz over ~3us of sustained execution) is
already warm when the first projections land; init memsets ride Pool's
bitcast-f32 path to keep the DVE queue clear for the critical kT copies.

Single-shot cost-model timeline: 63.3us (baseline f32r kernel: 84.5us).
Hardware: 68507ns/iter steady-state (For_i K-delta, K=512 vs 8192, medians)
vs the baseline's 140012ns; rel err 6.4e-3 vs the fp32 reference (bf16 path).
GOTCHAS (hardware-verified): GPSIMD cannot touch PSUM; PSUM accesses must be
4B-aligned (bf16 odd-element offsets are not); gpsimd memset on bf16 writes
32-bit patterns (use DVE); engine partition starts must be multiples of 32.
"""
import numpy as np
import ml_dtypes
from contextlib import ExitStack

import concourse.bass as bass
import concourse.tile as tile
from concourse import bacc, mybir
from concourse.bass_utils import run_bass_kernel_spmd

f32 = mybir.dt.float32
bf16 = mybir.dt.bfloat16
i32 = mybir.dt.int32
AF = mybir.ActivationFunctionType
ET = mybir.EngineType
bfloat16 = ml_dtypes.bfloat16

B, S, DE, DK, DV = 4, 4096, 512, 64, 64
QT = 512                 # queries per group
NT = S // QT             # 8 s/q tiles per batch
NG = 4                   # groups (q-tiles) per core
NCH = DE // 128          # 4 contraction chunks
TW = NCH * QT            # tile width in sbuf cols (2048)
TQ = [[0, 2, 5, 7], [1, 3, 4, 6]]   # parity -> group -> q_tile index

# palette bases (element cols): [drop(896) | keep(896) | tri(896)]
# window for block b is base - 128*b, so bases sit +384 into each region.
PAL_DROP = 384
PAL_KEEP = 896 + 384
PAL_TRI = 2 * 896 + 384
PAL_W = 3 * 896


def build(kiter: int = 1):
    nc = bacc.Bacc("TRN2", target_bir_lowering=False, debug=False)

    xt_d = nc.dram_tensor("xt", [NT, 128, TW], bf16, kind="ExternalInput").ap()
    xq_d = nc.dram_tensor("xq", [NG, 128, TW], bf16, kind="ExternalInput").ap()
    wb_d = nc.dram_tensor("wb", [128, NCH * (128 + DK)], bf16,
                          kind="ExternalInput").ap()
    moff_d = nc.dram_tensor("moff", [1, 8], i32, kind="ExternalInput").ap()
    tri_d = nc.dram_tensor("tri", [128, 896], bf16, kind="ExternalInput").ap()
    ident_d = nc.dram_tensor("ident", [66, 66], bf16, kind="ExternalInput").ap()
    out_d = nc.dram_tensor("out", [NG, 65, QT], bf16, kind="ExternalOutput").ap()

    with tile.TileContext(nc) as tc, ExitStack() as ctx:

        def body():
            per = ctx.enter_context(tc.tile_pool(name="persist", bufs=1))
            pkv_pool = ctx.enter_context(tc.tile_pool(name="pkv", bufs=2, space="PSUM"))
            pq_pool = ctx.enter_context(tc.tile_pool(name="pq", bufs=2, space="PSUM"))
            ps_pool = ctx.enter_context(tc.tile_pool(name="ps", bufs=2, space="PSUM"))

            exp_pool = ctx.enter_context(tc.tile_pool(name="exp", bufs=8))
            fin_pool = ctx.enter_context(tc.tile_pool(name="fin", bufs=2))

            xts = per.tile([128, NT * TW], bf16)           # x^T, tile-major
            xqs = per.tile([128, NG * TW], bf16)           # x^T own q-tiles
            wb = per.tile([128, NCH * (128 + DK)], bf16)   # [wkv(4x128)|wq(4x64)]
            ident = per.tile([66, 66], bf16)
            pal = per.tile([128, PAL_W], bf16)
            kT = per.tile([128, S], bf16)   # rows 0:64 and 64:128 both hold k^T
            vaug = per.tile([128, (S // 128) * 66], bf16)  # 32 x [128,66(65 used)]
            vts = per.tile([66, 2 * QT], bf16)             # ping-pong v rows
            qTg = per.tile([128, NG * QT], bf16)  # duplicated rows like kT
            mofft = per.tile([1, 8], i32)

            # DMA emission order == transfer priority. moff rides the cheap
            # Pool queue; xq/xt round-robin between SP and ACT queues so the
            # HWDGE interleaves them in just-in-time order.
            # DMA issues cost ~1.5us of sequencer time each. SP and ACT
            # alternate so HWDGE interleaves transfers in need order, but ACT
            # gets only the first 4 issues so its queue is clear well before
            # the first exp dispatch (~10.5us); SP absorbs the rest.
            nc.sync.dma_start(wb[:], wb_d[:])
            nc.scalar.dma_start(xqs[:, bass.ts(0, TW)], xq_d[0])
            nc.sync.dma_start(xts[:, bass.ts(0, TW)], xt_d[0])
            nc.scalar.dma_start(xts[:, bass.ts(1, TW)], xt_d[1])
            nc.sync.dma_start(xqs[:, bass.ts(1, TW)], xq_d[1])
            nc.scalar.dma_start(xts[:, bass.ts(2, TW)], xt_d[2])
            nc.sync.dma_start(pal[:, 2 * 896:3 * 896], tri_d[:])
            nc.scalar.dma_start(xts[:, bass.ts(3, TW)], xt_d[3])
            nc.sync.dma_start(xts[:, bass.ts(4, TW)], xt_d[4])
            nc.sync.dma_start(xqs[:, bass.ts(2, TW)], xq_d[2])
            nc.sync.dma_start(xts[:, bass.ts(5, TW)], xt_d[5])
            nc.sync.dma_start(xts[:, bass.ts(6, TW)], xt_d[6])
            nc.sync.dma_start(xts[:, bass.ts(7, TW)], xt_d[7])
            nc.sync.dma_start(xqs[:, bass.ts(3, TW)], xq_d[3])
            nc.gpsimd.dma_start(mofft[:], moff_d[:])
            nc.gpsimd.dma_start(ident[:], ident_d[:])
            ONE2 = 1.0019378662109375   # f32 with bits 0x3F803F80 = bf16 (1,1)
            nc.gpsimd.memset(pal.bitcast(f32)[:, 0:448], 0.0)
            nc.gpsimd.memset(pal.bitcast(f32)[:, 448:896], ONE2)
            nc.gpsimd.memset(vts.bitcast(f32)[64:66, :], 0.0)
            nc.gpsimd.memset(vts.bitcast(f32)[64:65, :], ONE2)

            # mask palette base offsets, loaded JIT in pairs (2 per group)
            mv = [None] * 8

            def load_mv(g):
                for j in range(2):
                    mv[2 * g + j] = nc.values_load(
                        mofft[0:1, 2 * g + j:2 * g + j + 1].to_broadcast((1, 1)),
                        engines=[ET.DVE], min_val=PAL_DROP, max_val=PAL_TRI,
                        skip_runtime_bounds_check=True)

            # ---- projections ------------------------------------------------
            def q_proj(g):
                pq_t = pq_pool.tile([65, QT], f32, tag="pqo")
                pq = pq_t[0:64, :]
                for c in range(NCH):
                    nc.tensor.matmul(pq[:], wb[:, NCH * 128 + c * DK:
                                                NCH * 128 + (c + 1) * DK],
                                     xqs[:, g * TW + c * QT: g * TW + (c + 1) * QT],
                                     start=(c == 0), stop=(c == NCH - 1))
                nc.vector.tensor_copy(qTg[0:64, bass.ts(g, QT)], pq[:])
                nc.vector.tensor_copy(qTg[64:128, bass.ts(g, QT)],
                                      qTg[0:64, bass.ts(g, QT)])

            def t0c(t, c, h):
                # 256-col half h of contraction chunk c within tile t
                base = t * TW + c * QT + h * 256
                return slice(base, base + 256)

            # k^T and v rows for one s-tile (phase 1: matmuls + copies)
            def kv_mm(t):
                pkv = pkv_pool.tile([128, QT], f32, tag="pkvt")
                for c in range(NCH):
                    nc.tensor.matmul(pkv[:], wb[:, bass.ts(c, 128)],
                                     xts[:, t * TW + c * QT: t * TW + (c + 1) * QT],
                                     start=(c == 0), stop=(c == NCH - 1))
                nc.vector.tensor_copy(kT[0:64, bass.ts(t, QT)], pkv[0:64, :])
                nc.vector.tensor_copy(kT[64:128, bass.ts(t, QT)],
                                      kT[0:64, bass.ts(t, QT)])
                vt = vts[:, (t % 2) * QT:(t % 2) * QT + QT]
                nc.vector.tensor_copy(vt[0:64, :], pkv[64:128, :])

            # phase 2 (emitted a couple of score-pairs later so the vts copy
            # latency hides behind score matmuls): v^T blocks into vaug
            def kv_tr(t):
                vt = vts[:, (t % 2) * QT:(t % 2) * QT + QT]
                # 66-element block stride keeps bf16 PSUM writes 4B-aligned
                pvt = pkv_pool.tile([128, 4 * 66], bf16, tag="pkvt")
                for blk in range(4):
                    nc.tensor.transpose(pvt[:, bass.ts(blk, 66)],
                                        vt[:, bass.ts(blk, 128)],
                                        ident[:])
                nc.vector.tensor_copy(vaug[:, t * 4 * 66:(t + 1) * 4 * 66], pvt[:])

            # PE pstate warmup: dummy matmuls on scratch during the DMA wait
            scr = per.tile([128, 384], bf16)
            nc.vector.memset(scr[:], 0.0)
            psd = ps_pool.tile([128, 2 * QT], f32, name="ps")
            NWARM = 16
            for w in range(NWARM):
                nc.tensor.matmul(psd[:, 0:256], scr[:, 0:128], scr[:, 128:384],
                                 start=(w == 0), stop=(w == NWARM - 1))

            # head: project q-tile 0, then kv tile 0 in two 256-col halves so
            # the first score pair (which only needs kv 0:256) fires earlier.
            q_proj(0)
            pkv0 = pkv_pool.tile([128, QT], f32, tag="pkvt")
            for h in range(2):
                cs = slice(h * 256, h * 256 + 256)
                for c in range(NCH):
                    nc.tensor.matmul(pkv0[:, cs],
                                     wb[:, bass.ts(c, 128)],
                                     xts[:, t0c(0, c, h)],
                                     start=(c == 0), stop=(c == NCH - 1))
                nc.vector.tensor_copy(kT[0:64, h * 256:h * 256 + 256],
                                      pkv0[0:64, cs])
                nc.vector.tensor_copy(kT[64:128, h * 256:h * 256 + 256],
                                      kT[0:64, h * 256:h * 256 + 256])
                nc.vector.tensor_copy(vts[0:64, h * 256:h * 256 + 256],
                                      pkv0[64:128, cs])
            load_mv(0)

            # flat 40-pair pipeline across all groups; masked pairs mid-group
            # (g3: early) so their deferred AVs drain inside later flushes.
            def group_order(g):
                unm = list(range(4 * g))
                msk = list(range(4 * g, 4 * g + 4))
                if not unm:
                    return msk
                if g == NG - 1:
                    return unm[:2] + msk + unm[2:]
                return unm[:-1] + msk + unm[-1:]

            flat = [(g, pi) for g in range(NG) for pi in group_order(g)]

            # projections are back-loaded into late (exp-bound) groups so the
            # PE has slack in early groups where exp is already saturated.
            sched = {
                0: [lambda: kv_tr(0)],
                1: [lambda: kv_mm(1)],
                2: [lambda: kv_tr(1)],
                3: [lambda: q_proj(1), lambda: load_mv(1)],
                5: [lambda: kv_mm(2)],
                6: [lambda: kv_tr(2)],
                7: [lambda: kv_mm(3)],
                9: [lambda: kv_tr(3)],
                10: [lambda: q_proj(2), lambda: load_mv(2)],
                14: [lambda: kv_mm(4)],
                16: [lambda: kv_tr(4), lambda: kv_mm(5)],
                18: [lambda: kv_tr(5)],
                20: [lambda: q_proj(3), lambda: load_mv(3)],
                22: [lambda: kv_mm(6)],
                24: [lambda: kv_tr(6), lambda: kv_mm(7)],
                26: [lambda: kv_tr(7)],
            }

            po = {}
            av_emitted = {g: 0 for g in range(NG)}
            deferred = []
            pending = None

            def finalize(g):
                # stage the unnormalized [65,512] group result and DMA it out
                ofin = fin_pool.tile([65, QT], bf16)
                nc.vector.tensor_copy(ofin[:], po[g][:])
                nc.sync.dma_start(out_d[g], ofin[:])

            def emit_av(g, pi, em):
                n_av = 2 * (4 * g + 4)      # total AV matmuls for this group
                for half in range(2):
                    sb = 2 * pi + half
                    nc.tensor.matmul(po[g][:], vaug[:, sb * 66:sb * 66 + 65],
                                     em[:, bass.ts(half, QT)],
                                     start=(av_emitted[g] == 0),
                                     stop=(av_emitted[g] == n_av - 1))
                    av_emitted[g] += 1
                if av_emitted[g] == n_av:
                    finalize(g)

            def flush():
                nonlocal pending
                if pending is None:
                    return
                # deferred masked AVs from earlier flushes are ready now
                for dg, dpi, dem in deferred:
                    emit_av(dg, dpi, dem)
                deferred.clear()
                ps, g, pi = pending
                pending = None
                em = exp_pool.tile([128, 2 * QT], bf16)
                nc.scalar.activation(em[:], ps[:], AF.Exp)
                if pi >= 4 * g:   # masked pair: multiplicative palette mask
                    for half in range(2):
                        rel = 2 * (pi - 4 * g) + half
                        off = mv[2 * g + rel // 4] - 128 * (rel % 4)
                        nc.vector.tensor_mul(
                            em[:, bass.ts(half, QT)],
                            em[:, bass.ts(half, QT)],
                            pal[:, bass.ds(off, QT)])
                    deferred.append((g, pi, em))
                else:
                    emit_av(g, pi, em)

            for idx, (g, pi) in enumerate(flat):
                if pi == group_order(g)[0]:
                    po[g] = pq_pool.tile([65, QT], f32, tag="pqo", name="po")
                ps = ps_pool.tile([128, 2 * QT], f32)
                for half in range(2):
                    sb = 2 * pi + half
                    rows = slice(64 * half, 64 * half + 64)
                    nc.tensor.matmul(ps[:, bass.ts(half, QT)],
                                     kT[rows, bass.ts(sb, 128)],
                                     qTg[rows, bass.ts(g, QT)],
                                     start=True, stop=True,
                                     tile_position=(64 * half, 0))
                flush()
                pending = (ps, g, pi)
                for thunk in sched.get(idx, []):
                    thunk()
            flush()
            for dg, dpi, dem in deferred:
                emit_av(dg, dpi, dem)
            deferred.clear()

        if kiter == 1:
            body()
        else:
            with tc.For_i(0, kiter, 1, staggered_reset=True):
                body()

    nc.compile()
    return nc


def _tile_cols(a):
    """[512, n*512] (d_e, cols) -> [n, 128, 4*512] tile-major host layout."""
    de, w = a.shape
    n = w // QT
    # out[t, p, c*QT + s] = a[c*128 + p, t*QT + s]
    return np.ascontiguousarray(
        a.reshape(NCH, 128, n, QT).transpose(2, 1, 0, 3).reshape(n, 128, NCH * QT))


def make_inputs(x, Wq, Wk, Wv):
    """Per-core input maps. x:[B,S,DE] f32; W*: [DE,64] f32."""
    wkv = np.concatenate([Wk, Wv], axis=1).astype(np.float32)          # [512,128]
    wqs = (Wq / np.float32(np.sqrt(DK))).astype(np.float32)            # [512,64]
    # weights chunk-major: [128, c*width + j] = W[c*128 + p, j]
    wkv_h = np.ascontiguousarray(
        wkv.reshape(NCH, 128, 128).transpose(1, 0, 2).reshape(128, NCH * 128))
    wq_h = np.ascontiguousarray(
        wqs.reshape(NCH, 128, DK).transpose(1, 0, 2).reshape(128, NCH * DK))
    wb_h = np.concatenate([wkv_h, wq_h], axis=1).astype(bfloat16)
    ident = np.eye(66, dtype=bfloat16)
    tri = (np.arange(896)[None, :] >= np.arange(128)[:, None] + 384).astype(bfloat16)
    in_maps = []
    for core in range(8):
        b, p = core // 2, core % 2
        xt = np.ascontiguousarray(x[b].T, dtype=np.float32)            # [512, 4096]
        cols = np.concatenate([np.arange(t * QT, (t + 1) * QT) for t in TQ[p]])
        moff = np.zeros((1, 8), dtype=np.int32)
        for g in range(NG):
            t = TQ[p][g]
            for jr in range(2):
                j = 2 * g + jr
                if j < t:
                    moff[0, 2 * g + jr] = PAL_KEEP
                elif j == t:
                    moff[0, 2 * g + jr] = PAL_TRI
                else:
                    moff[0, 2 * g + jr] = PAL_DROP
        in_maps.append(dict(xt=_tile_cols(xt).astype(bfloat16),
                            xq=_tile_cols(xt[:, cols]).astype(bfloat16),
                            wb=wb_h, moff=moff, tri=tri, ident=ident))
    return in_maps


def assemble(results):
    out = np.empty((B, S, DV), dtype=np.float32)
    for core in range(8):
        b, p = core // 2, core % 2
        o = results[core]["out"].astype(np.float32)   # [NG, 65, QT] bf16
        for g in range(NG):
            t = TQ[p][g]
            out[b, t * QT:(t + 1) * QT, :] = (o[g][0:64, :] / o[g][64:65, :]).T
    return out


_cache = {}


def _get_nc(kiter=1):
    if kiter not in _cache:
        _cache[kiter] = build(kiter)
    return _cache[kiter]


def run(x, Wq, Wk, Wv, kiter=1):
    nc = _get_nc(kiter)
    in_maps = make_inputs(x, Wq, Wk, Wv)
    res = run_bass_kernel_spmd(nc, in_maps, list(range(8)))
    return assemble(res.results)


def kernel(x, Wq, Wk, Wv):
    x = np.asarray(x, dtype=np.float32)
    return run(x, np.asarray(Wq, np.float32), np.asarray(Wk, np.float32),
               np.asarray(Wv, np.float32))
